# revision 43
# baseline (speedup 1.0000x reference)
"""Trainium2 Bass kernel for nn_MixedLipMlp (soft-MoE MLP with Lipschitz gate).

Strategy: data-parallel over batch B=4096 across 8 NeuronCores (512 rows each,
expert weights + gate replicated). Everything on-chip is feature-major
(features on partitions, batch on the free dim): activations stream as matmul
rhs, weights as lhsT — no transposes anywhere.

v2 redesign vs the f32r baseline (151us):
  * Lipschitz row-scaling of the gate weights is folded on the HOST (it is
    weight-only math), deleting the on-chip abs/rowsum/softplus/recip chain
    and two ACT table loads from the serial head.
  * The whole MoE runs in bf16: same PE throughput as f32r (1 cyc/row), but
    half the HBM traffic, 2x DVE throughput on the per-expert coefficient
    scaling (all-bf16 tensor_tensor), and column-group tile_position support
    which f32r lacks.
  * coefficient broadcast to 128 partitions is one K=8 matmul per expert with
    a host-built selector matrix (no row staging DMAs).
  * z is coefficient-scaled per expert-PAIR ([zsf_even; zsf_odd] stacked in
    one 128-row tile), so the z-feature matmuls are plain full-K matmuls.
  * layer 2 (576 -> 12) runs 4 experts concurrently in 32-column PE groups,
    and the coefficient mix over experts is done by the PE: one
    strip-broadcast matmul per 4-expert round (mixQ), a DVE multiply, and a
    host-built partition-reduction matmul, accumulated with the bias term in
    a single PSUM bank.
  * the PE is pre-warmed with filler matmuls during the DMA preamble so the
    HAM clock gate reaches 8/8 before the gate phase.
  elu(y) = min(exp(y)-1, relu(y)) — 2 ACT + 1 fused DVE op per tile.
"""

import os
import sys

if "/opt/trn_rl_repo" not in sys.path:
    sys.path.insert(0, "/opt/trn_rl_repo")

# recover cleanly if a previous process left the NeuronCores wedged
os.environ.setdefault("NEURON_RT_RESET_CORES", "1")

import numpy as np
import ml_dtypes

BF16NP = ml_dtypes.bfloat16

# Problem dimensions (hardcoded; must match the grader's setup_inputs()).
B = 4096
NCORES = 8
BS = B // NCORES  # 512 batch rows per core = matmul free dim
LATENT = 64
INPUT_SIZE = 256
IN_DIM = LATENT + INPUT_SIZE  # 320
HIDDEN = 512
ACTIONS = 12
E = 8
GATE_H = 128
INTER = HIDDEN + LATENT  # 576
NPAIR = E // 2
L2W = 32  # l2 output padded 12 -> 32 (one PE column group)

NK0 = 2  # layer0: c has 256 rows = 2 slabs
NK12 = 4  # layers1,2: h has 512 rows = 4 slabs
N_M = HIDDEN // 128  # 4 output m-tiles for layers 0/1

N_FILL_PRE = 4  # warm-up matmuls during the DMA preamble
N_FILL_GAP = 2  # fillers between gate stages (keep HAM from re-throttling)

TRACE = False
DEBUG_TAPS = False  # extra DMA outputs of intermediates for bisection
LAST_EXEC_NS = None
LAST_RESULTS = None


def _build_nc():
    import concourse.mybir as mybir
    from concourse import bacc
    from concourse.tile import TileContext
    from contextlib import ExitStack

    dt = mybir.dt
    F32 = dt.float32
    F32R = dt.float32r
    BF = dt.bfloat16
    AF = mybir.ActivationFunctionType
    OP = mybir.AluOpType

    nc = bacc.Bacc("TRN2", target_bir_lowering=False)

    # ---- DRAM I/O. Small tensors are host-bundled into per-queue "cat"
    # tensors so each lands in ONE serialized-ring DMA instead of ~8 -------
    # gcat: gw0 slab0 (rows 0:64) | gw0 slab1 | gw0 slab2 | gw1 | gw2 |
    #       gb0 | gb1 | gb2(rows 0:8)    (gate weights host-lip-folded)
    d_gcat = nc.dram_tensor("gcat", [128, 523], F32R, kind="ExternalInput")
    # xcat: zT (rows 0:64) | cT[:128] | cT[128:]  (f32 gate inputs)
    d_xcat = nc.dram_tensor("xcat", [128, 3 * BS], F32R, kind="ExternalInput")
    # bcat: xz2b | xcb0 | xcb1 | selt(rows 0:8) | redt   (bf16)
    d_bcat = nc.dram_tensor("bcat", [128, 3 * BS + (E + 2) * 128 + L2W], BF,
                            kind="ExternalInput")
    # w0cat: w0z | b0(rows 0:8); w12cat: w1z | b1 | w2z | b2(rows 0:8)
    d_w0cat = nc.dram_tensor("w0cat", [128, NPAIR * HIDDEN + HIDDEN], BF,
                             kind="ExternalInput")
    d_w12cat = nc.dram_tensor(
        "w12cat", [128, NPAIR * HIDDEN + HIDDEN + NPAIR * L2W + L2W], BF,
        kind="ExternalInput")
    d_w0h = nc.dram_tensor("w0h", [E, 128, NK0 * HIDDEN], BF, kind="ExternalInput")
    d_w1h = nc.dram_tensor("w1h", [E, 128, NK12 * HIDDEN], BF, kind="ExternalInput")
    d_w2h = nc.dram_tensor("w2h", [E, 128, NK12 * L2W], BF, kind="ExternalInput")
    d_out = nc.dram_tensor("outT", [ACTIONS, BS], F32, kind="ExternalOutput")
    d_dbg = {}
    if DEBUG_TAPS:
        for nm, shp in [("coeff", [E, BS]), ("h0", [HIDDEN, BS]),
                        ("h1", [HIDDEN, BS]), ("mixq0", [128, BS]),
                        ("prod0", [128, BS]), ("prod1", [128, BS]),
                        ("bce0", [128, BS]), ("zs0", [128, BS])]:
            d_dbg[nm] = nc.dram_tensor(f"dbg_{nm}", shp, BF,
                                       kind="ExternalOutput")

    mm = nc.tensor.matmul

    with TileContext(nc) as tc:
        with ExitStack() as ctx:
            pers = ctx.enter_context(tc.tile_pool(name="pers", bufs=1))
            sca = ctx.enter_context(tc.tile_pool(name="sca", bufs=20))
            etmp = ctx.enter_context(tc.tile_pool(name="etmp", bufs=6))

            # ---- constants ------------------------------------------------
            ones_f = pers.tile([128, 128], F32, tag="ones_f")
            nc.vector.memset(ones_f, 1.0)
            ones_b = pers.tile([128, BS], BF, tag="ones_b")
            nc.vector.memset(ones_b, 1.0)

            # ---- DMA issue. Both rings transfer serially per-queue; the
            # bundled tensors make the gate set land in 1 transfer per ring.
            gcat = pers.tile([128, 523], F32R, tag="gcat")
            nc.sync.dma_start(out=gcat, in_=d_gcat[:, :])
            xcat = pers.tile([128, 3 * BS], F32R, tag="xcat")
            nc.gpsimd.dma_start(out=xcat, in_=d_xcat[:, :])
            gw0t = [gcat[0:64, 0:128], gcat[:, 128:256], gcat[:, 256:384]]
            gw1t = gcat[:, 384:512]
            gw2t = gcat[:, 512:520]
            gb0t = gcat[:, 520:521].bitcast(F32)
            gb1t = gcat[:, 521:522].bitcast(F32)
            gb2t = gcat[0:E, 522:523].bitcast(F32)
            xz = xcat[0:LATENT, 0:BS]
            xc0 = xcat[:, BS : 2 * BS]
            xc1 = xcat[:, 2 * BS : 3 * BS]

            bcat = pers.tile([128, 3 * BS + (E + 2) * 128 + L2W], BF,
                             tag="bcat")
            nc.gpsimd.dma_start(out=bcat, in_=d_bcat[:, :])
            xz2b = bcat[:, 0:BS]
            xcb = [bcat[:, BS : 2 * BS], bcat[:, 2 * BS : 3 * BS]]
            selt = bcat[0:E, 3 * BS : 3 * BS + (E + 2) * 128]
            redt = bcat[:, 3 * BS + (E + 2) * 128 :]

            w0cat = pers.tile([128, NPAIR * HIDDEN + HIDDEN], BF, tag="w0cat")
            nc.sync.dma_start(out=w0cat, in_=d_w0cat[:, :])
            w0z = w0cat[:, 0 : NPAIR * HIDDEN]
            b0b = w0cat[0:E, NPAIR * HIDDEN :]
            w0h = []
            for e in range(E):
                t = pers.tile([128, NK0 * HIDDEN], BF, tag=f"w0h{e}")
                eng = nc.sync if e % 2 == 0 else nc.gpsimd
                eng.dma_start(out=t, in_=d_w0h[e, :, :])
                w0h.append(t)

            w12cat = pers.tile(
                [128, NPAIR * HIDDEN + HIDDEN + NPAIR * L2W + L2W], BF,
                tag="w12cat")
            nc.gpsimd.dma_start(out=w12cat, in_=d_w12cat[:, :])
            w1z = w12cat[:, 0 : NPAIR * HIDDEN]
            b1b = w12cat[0:E, NPAIR * HIDDEN : NPAIR * HIDDEN + HIDDEN]
            w2z = w12cat[:, NPAIR * HIDDEN + HIDDEN :
                         NPAIR * HIDDEN + HIDDEN + NPAIR * L2W]
            b2p = w12cat[0:E, NPAIR * HIDDEN + HIDDEN + NPAIR * L2W :]
            w1h = []
            for e in range(E):
                t = pers.tile([128, NK12 * HIDDEN], BF, tag=f"w1h{e}")
                nc.gpsimd.dma_start(out=t, in_=d_w1h[e, :, :])
                w1h.append(t)
            w2h = []
            for e in range(E):
                t = pers.tile([128, NK12 * L2W], BF, tag=f"w2h{e}")
                nc.gpsimd.dma_start(out=t, in_=d_w2h[e, :, :])
                w2h.append(t)

            # ---- PSUM: ONE pool for the whole kernel (8 banks): a 4-deep
            # "g" ring shared by the gate stages and all MoE accumulators
            # (so L0 inherits the gate's early-released banks), "sum", and a
            # 3-deep "bc" ring (fillers + per-expert broadcast matmuls)
            head_ctx = tc.tile_pool(name="ps_all", bufs=1, space="PSUM")
            ps_head = head_ctx.__enter__()

            warm = [ps_head.tile([128, BS], F32, tag="bc", bufs=3,
                                 name=f"warm{i}") for i in range(2)]

            def filler(n, nfree=BS):
                for i in range(n):
                    mm(warm[i % 2][:, :nfree], ones_b[:, :128],
                       ones_b[:, :nfree], start=True, stop=True)

            # prime the ACT table (exp_and_others: Exp/Relu/Copy) while the
            # DMAs stream, instead of on the gate critical path
            prime = etmp.tile([1, 1], F32, tag="prime")
            nc.scalar.activation(out=prime, in_=ones_f[0:1, 0:1], func=AF.Exp,
                                 bias=0.0, scale=1.0)

            filler(N_FILL_PRE)

            def gap_filler():
                filler(N_FILL_GAP, nfree=512)

            def elu_from_psum(ps, bias, out_tile, tdt=BF, dve_relu=False):
                # elu = min(exp(y)-1, relu(y)); bias folded into the ACT ops
                n = ps.shape[1]
                ex = etmp.tile([ps.shape[0], n], tdt, tag="elu_exp")
                nc.scalar.activation(out=ex, in_=ps, func=AF.Exp, bias=bias,
                                     scale=1.0)
                rl = etmp.tile([ps.shape[0], n], tdt, tag="elu_relu")
                if dve_relu:
                    nc.vector.tensor_scalar(out=rl, in0=ps, scalar1=0.0,
                                            scalar2=None, op0=OP.max)
                else:
                    nc.scalar.activation(out=rl, in_=ps, func=AF.Relu,
                                         bias=bias, scale=1.0)
                nc.vector.scalar_tensor_tensor(
                    out=out_tile, in0=ex, scalar=1.0, in1=rl,
                    op0=OP.subtract, op1=OP.min,
                )

            # gate, processed in two independent 256-column streams so the
            # next stage's matmul starts as soon as its half's elu is done
            HB = BS // 2
            cols = [slice(0, HB), slice(HB, BS)]
            rhs0 = [xz[0:LATENT, :], xc0, xc1]
            h0g = pers.tile([GATE_H, BS], F32R, tag="h0g")
            h1g = pers.tile([GATE_H, BS], F32R, tag="h1g")
            expl = pers.tile([E, BS], F32, tag="expl")
            ps_g0, ps_g1, ps_g2 = [], [], []
            for h in range(2):
                ps = ps_head.tile([GATE_H, BS], F32, tag="g", bufs=4,
                                  name=f"psg0_{h}")
                for k in range(3):
                    mm(ps[:, :HB], gw0t[k], rhs0[k][:, cols[h]],
                       start=(k == 0), stop=(k == 2))
                ps_g0.append(ps)
            for h in range(2):
                elu_from_psum(ps_g0[h][:, :HB], gb0t, h0g[:, cols[h]],
                              tdt=F32)
            gap_filler()
            for h in range(2):
                ps = ps_head.tile([GATE_H, BS], F32, tag="g", bufs=4,
                                  name=f"psg1_{h}")
                mm(ps[:, :HB], gw1t, h0g[:, cols[h]], start=True, stop=True)
                ps_g1.append(ps)
            for h in range(2):
                elu_from_psum(ps_g1[h][:, :HB], gb1t, h1g[:, cols[h]],
                              tdt=F32)
            gap_filler()
            for h in range(2):
                ps = ps_head.tile([GATE_H, BS], F32, tag="g", bufs=4,
                                  name=f"psg2_{h}")
                mm(ps[:E, :HB], gw2t, h1g[:, cols[h]], start=True, stop=True)
                ps_g2.append(ps)
                # softmax numerator: logits are Lipschitz-bounded, so no max
                # subtraction: expl = exp(logits + gb2)
                nc.scalar.activation(out=expl[:, cols[h]],
                                     in_=ps_g2[h][:E, :HB], func=AF.Exp,
                                     bias=gb2t, scale=1.0)
            gap_filler()

            # denominator, replicated to all 8 partitions directly by an
            # all-ones K=8 M=8 fp32 matmul; fast-approx reciprocal (~2e-6)
            ps_sum = ps_head.tile([E, BS], F32, tag="sum")
            mm(ps_sum, ones_f[:E, :E], expl, start=True, stop=True)
            rec8 = pers.tile([E, BS], F32, tag="rec8")
            nc.vector.reciprocal_approx_fast(out=rec8, in_=ps_sum)
            coeffT = pers.tile([E, BS], BF, tag="coeffT")
            nc.vector.tensor_mul(coeffT, expl, rec8)
            gap_filler()

            # per-expert coefficient broadcast: one K=8 matmul per expert
            # with a host-built one-hot selector as lhsT; evacuation split
            # between ACT and DVE. Scaled-z pair tiles built as each pair's
            # coefficients land.
            bcE = []
            zsf = []
            for e in range(E):
                pb = ps_head.tile([128, BS], F32, tag="bc", bufs=3,
                                  name=f"pbc{e}")
                mm(pb, selt[:, 128 * e : 128 * (e + 1)], coeffT,
                   start=True, stop=True)
                t = pers.tile([128, BS], BF, tag=f"bcE{e}")
                if e % 2 == 0:
                    nc.scalar.activation(out=t, in_=pb, func=AF.Copy,
                                         bias=0.0, scale=1.0)
                else:
                    nc.vector.tensor_copy(out=t, in_=pb)
                bcE.append(t)
                if e % 2 == 1:
                    p = e // 2
                    zp = pers.tile([128, BS], BF, tag=f"zsf{p}")
                    nc.vector.tensor_mul(
                        zp[:LATENT, :], xz2b[:LATENT, :], bcE[e - 1][:LATENT, :]
                    )
                    nc.vector.tensor_mul(
                        zp[LATENT:, :], xz2b[LATENT:, :], bcE[e][LATENT:, :]
                    )
                    zsf.append(zp)


            def moe_layer(wz, wh, hs_src, nk, bsb, psl):
                # z pass: expert pairs stacked into full-K matmuls
                for p in range(NPAIR):
                    for m in range(N_M):
                        mm(psl[m],
                           wz[:, p * HIDDEN + 128 * m : p * HIDDEN + 128 * (m + 1)],
                           zsf[p], start=(p == 0), stop=False)
                # h pass, k-slab-outer: slab ki only needs the previous
                # layer's m-tile ki, so the first matmuls start as soon as
                # the first epilogue tile lands instead of the last
                for ki in range(nk):
                    hs = []
                    for e in range(E):
                        t = sca.tile([128, BS], BF, tag="s", name=f"s{e}_{ki}")
                        nc.vector.tensor_mul(t, hs_src[ki], bcE[e])
                        hs.append(t)
                    for e in range(E):
                        for m in range(N_M):
                            mm(psl[m],
                               wh[e][:, ki * HIDDEN + 128 * m :
                                     ki * HIDDEN + 128 * (m + 1)],
                               hs[e], start=False, stop=False)
                # bias: out += coeff @ b (K=8 matmul closes each bank)
                for m in range(N_M):
                    mm(psl[m], bsb[:, 128 * m : 128 * (m + 1)], coeffT,
                       start=False, stop=True)

            # layer 0: (320 -> 512), elu
            ps_l0 = [ps_head.tile([128, BS], F32, tag="g", bufs=4,
                                  name=f"psl0_{m}") for m in range(N_M)]
            moe_layer(w0z, w0h, xcb, NK0, b0b, ps_l0)
            h0m = []
            for m in range(N_M):
                t = pers.tile([128, BS], BF, tag=f"h0m{m}")
                elu_from_psum(ps_l0[m], 0.0, t)
                h0m.append(t)

            # layer 1: (576 -> 512), elu
            ps_l1 = [ps_head.tile([128, BS], F32, tag="sum", name="psl1_0")]
            ps_l1 += [ps_head.tile([128, BS], F32, tag="bc", bufs=3,
                                   name=f"psl1_{m}") for m in range(1, N_M)]
            moe_layer(w1z, w1h, h0m, NK12, b1b, ps_l1)
            h1m = []
            for m in range(N_M):
                t = pers.tile([128, BS], BF, tag=f"h1m{m}")
                # relu alternates ACT/DVE so neither engine serializes the
                # epilogue chain that gates layer 2's h-slab matmuls
                elu_from_psum(ps_l1[m], 0.0, t, dve_relu=(m % 2 == 0))
                h1m.append(t)

            # ---- layer 2: (576 -> 12), 4 experts per PE column-group round
            # u_r strips hold per-expert unscaled outputs; mixQ_r broadcasts
            # each expert's coefficients onto its strip; the strip-products
            # are partition-reduced by a host-built 0/1 matmul and
            # accumulated with the bias term in one PSUM bank.
            u_ps = [ps_head.tile([128, BS], F32, tag="g", bufs=4,
                                 name=f"u{r}") for r in range(2)]
            mixq = [ps_head.tile([128, BS], F32, tag="g", bufs=4,
                                 name=f"mixq{r}") for r in range(2)]
            red = ps_head.tile([L2W, BS], F32, tag="sum", name="red")
            mixq_sb = []
            for r in range(2):
                mm(mixq[r], selt[:, 128 * (E + r) : 128 * (E + r + 1)], coeffT,
                   start=True, stop=True)
                t = pers.tile([128, BS], BF, tag=f"mixq_sb{r}")
                nc.scalar.activation(out=t, in_=mixq[r], func=AF.Copy,
                                     bias=0.0, scale=1.0)
                mixq_sb.append(t)
            prods = []
            for r in range(2):
                u = u_ps[r]
                for j in range(4):
                    e = 4 * r + j
                    p, half = divmod(e, 2)
                    rows = slice(64 * half, 64 * half + 64)
                    # unscaled z here: the coefficient lands via mixq below.
                    # start=True per strip: the has_written clear is scoped
                    # to the instruction's own partition range
                    mm(u[32 * j : 32 * j + 32, :],
                       w2z[rows, L2W * p : L2W * (p + 1)],
                       xz2b[rows, :],
                       start=True, stop=False, skip_group_check=True,
                       tile_position=(64 * half, 32 * j))
                for ki in range(NK12):
                    for j in range(4):
                        e = 4 * r + j
                        mm(u[32 * j : 32 * j + 32, :],
                           w2h[e][:, L2W * ki : L2W * (ki + 1)],
                           h1m[ki],
                           start=False, stop=(ki == NK12 - 1),
                           skip_group_check=True,
                           tile_position=(0, 32 * j))
                if r == 0:
                    # bias goes first into the red bank; issued after the
                    # round-0 chains so its bank-reuse wait doesn't stall
                    # the PE FIFO
                    mm(red, b2p, coeffT, start=True, stop=False)
                pr = sca.tile([128, BS], BF, tag="prod", name=f"prod{r}")
                nc.vector.tensor_mul(pr, u, mixq_sb[r])
                prods.append(pr)
                mm(red, redt, pr, start=False, stop=(r == 1))

            out_sb = pers.tile([ACTIONS, BS], F32, tag="out_sb")
            nc.vector.tensor_copy(out=out_sb, in_=red[:ACTIONS, :])
            head_ctx.__exit__(None, None, None)

            nc.sync.dma_start(out=d_out[:, :], in_=out_sb)

            if DEBUG_TAPS:
                nc.sync.dma_start(out=d_dbg["coeff"][:, :], in_=coeffT)
                for m in range(N_M):
                    nc.sync.dma_start(
                        out=d_dbg["h0"][128 * m : 128 * (m + 1), :], in_=h0m[m])
                    nc.sync.dma_start(
                        out=d_dbg["h1"][128 * m : 128 * (m + 1), :], in_=h1m[m])
                nc.sync.dma_start(out=d_dbg["mixq0"][:, :], in_=mixq_sb[0])
                nc.sync.dma_start(out=d_dbg["prod0"][:, :], in_=prods[0])
                nc.sync.dma_start(out=d_dbg["prod1"][:, :], in_=prods[1])
                nc.sync.dma_start(out=d_dbg["bce0"][:, :], in_=bcE[0])
                nc.sync.dma_start(out=d_dbg["zs0"][:, :], in_=zsf[0])

    nc.finalize()
    return nc


_nc_cache = None


def _get_nc():
    global _nc_cache
    if _nc_cache is None:
        _nc_cache = _build_nc()
    return _nc_cache


def _patch_hook_errors():
    # exceptions inside the neuronx-cc hook are swallowed by the PJRT
    # plugin ("CallFunctionObjArgs: error condition"); print them here
    from concourse import bass2jax

    orig = bass2jax.neuronx_cc_hook
    if getattr(orig, "_err_patched", False):
        return

    def wrapped(*a, **k):
        import traceback

        try:
            return orig(*a, **k)
        except BaseException as e:
            print(getattr(e, "output", ""), file=sys.stderr)
            traceback.print_exc()
            raise

    wrapped._err_patched = True
    bass2jax.neuronx_cc_hook = wrapped


def _softplus(x):
    return np.logaddexp(0.0, x)


def _fold_gate(W, c):
    # LipschitzLinear: rows of W scaled so row-wise L1 norm <= softplus(c);
    # weight-only math, so folded on the host. Returns [in, out] f32.
    lip = _softplus(np.float64(c.reshape(-1)[0]))
    scale = np.minimum(lip / np.abs(np.float64(W)).sum(1), 1.0)
    return np.ascontiguousarray((W * scale[:, None].astype(np.float32)).T)


def _pack_z(w):
    # (E, in, out) -> [128, NPAIR*out]: per pair p, rows 0:64 = even expert's
    # z-slab, rows 64:128 = odd expert's
    z = w[:, :LATENT, :]
    top = z[0::2].transpose(1, 0, 2).reshape(LATENT, -1)
    bot = z[1::2].transpose(1, 0, 2).reshape(LATENT, -1)
    return np.concatenate([top, bot], axis=0)


def _pack_h(w, nk):
    # (E, in, out) -> (E, 128, nk*out) k-slab-major
    out = w.shape[2]
    return (w[:, LATENT:, :].reshape(E, nk, 128, out)
            .transpose(0, 2, 1, 3).reshape(E, 128, nk * out))


def _pad_w2(w):
    out = np.zeros((E, INTER, L2W), np.float32)
    out[:, :, :ACTIONS] = w
    return out


def _bf(a):
    return np.ascontiguousarray(np.asarray(a).astype(BF16NP))


def _consts():
    sel = np.zeros((E, (E + 2) * 128), BF16NP)
    for e in range(E):
        sel[e, 128 * e : 128 * (e + 1)] = 1
    for r in range(2):
        for j in range(4):
            base = 128 * (E + r) + 32 * j
            sel[4 * r + j, base : base + 32] = 1
    red = np.zeros((128, L2W), BF16NP)
    for j in range(4):
        red[32 * j + np.arange(L2W), np.arange(L2W)] = 1
    return sel, red


def kernel(**inputs):
    global LAST_EXEC_NS, LAST_RESULTS
    from concourse import bass_utils

    _patch_hook_errors()

    f = {k: np.ascontiguousarray(np.asarray(v, dtype=np.float32))
         for k, v in inputs.items()}

    sel, red = _consts()
    w2pad = _pad_w2(f["w2"])
    b2pad = np.zeros((E, L2W), np.float32)
    b2pad[:, :ACTIONS] = f["b2"]

    gw0T = _fold_gate(f["gw0"], f["gc0"])
    gcat = np.zeros((128, 523), np.float32)
    gcat[0:64, 0:128] = gw0T[0:64]
    gcat[:, 128:256] = gw0T[64:192]
    gcat[:, 256:384] = gw0T[192:320]
    gcat[:, 384:512] = _fold_gate(f["gw1"], f["gc1"])
    gcat[:, 512:520] = _fold_gate(f["gw2"], f["gc2"])
    gcat[:, 520] = f["gb0"]
    gcat[:, 521] = f["gb1"]
    gcat[0:E, 522] = f["gb2"]

    w0cat = np.zeros((128, NPAIR * HIDDEN + HIDDEN), BF16NP)
    w0cat[:, : NPAIR * HIDDEN] = _pack_z(f["w0"]).astype(BF16NP)
    w0cat[0:E, NPAIR * HIDDEN :] = f["b0"].astype(BF16NP)
    w12cat = np.zeros((128, NPAIR * HIDDEN + HIDDEN + NPAIR * L2W + L2W),
                      BF16NP)
    w12cat[:, : NPAIR * HIDDEN] = _pack_z(f["w1"]).astype(BF16NP)
    w12cat[0:E, NPAIR * HIDDEN : NPAIR * HIDDEN + HIDDEN] = (
        f["b1"].astype(BF16NP))
    w12cat[:, NPAIR * HIDDEN + HIDDEN :
           NPAIR * HIDDEN + HIDDEN + NPAIR * L2W] = (
        _pack_z(w2pad).astype(BF16NP))
    w12cat[0:E, NPAIR * HIDDEN + HIDDEN + NPAIR * L2W :] = (
        b2pad.astype(BF16NP))

    shared = {
        "gcat": gcat,
        "w0cat": np.ascontiguousarray(w0cat),
        "w12cat": np.ascontiguousarray(w12cat),
        "w0h": _bf(_pack_h(f["w0"], NK0)),
        "w1h": _bf(_pack_h(f["w1"], NK12)),
        "w2h": _bf(_pack_h(w2pad, NK12)),
    }
    in_maps = []
    for c in range(NCORES):
        sl = slice(c * BS, (c + 1) * BS)
        m = dict(shared)
        zT = np.ascontiguousarray(f["z"][sl].T)
        cT = np.ascontiguousarray(f["c"][sl].T)
        xcat = np.zeros((128, 3 * BS), np.float32)
        xcat[0:LATENT, 0:BS] = zT
        xcat[:, BS : 2 * BS] = cT[:128]
        xcat[:, 2 * BS :] = cT[128:]
        m["xcat"] = xcat
        bcat = np.zeros((128, 3 * BS + (E + 2) * 128 + L2W), BF16NP)
        zb = zT.astype(BF16NP)
        bcat[0:LATENT, 0:BS] = zb
        bcat[LATENT:128, 0:BS] = zb
        bcat[:, BS : 2 * BS] = cT[:128].astype(BF16NP)
        bcat[:, 2 * BS : 3 * BS] = cT[128:].astype(BF16NP)
        bcat[0:E, 3 * BS : 3 * BS + (E + 2) * 128] = sel
        bcat[:, 3 * BS + (E + 2) * 128 :] = red
        m["bcat"] = np.ascontiguousarray(bcat)
        in_maps.append(m)

    nc = _get_nc()
    res = bass_utils.run_bass_kernel_spmd(
        nc, in_maps, list(range(NCORES)), trace=TRACE
    )
    LAST_EXEC_NS = res.exec_time_ns
    LAST_RESULTS = res
    out = np.concatenate(
        [np.asarray(res.results[c]["outT"]).T for c in range(NCORES)], axis=0
    )
    return out
